# revision 1
# baseline (speedup 1.0000x reference)
"""CRF loss kernel for Trainium2 (8 NeuronCores, batch-sharded).

Forward algorithm in exp space: v_l = Etilde_l^T v_{l-1} per batch, where the
host folds chem cost, mask (masked step -> identity) and the e^{-kappa}
rescale into the shipped log-trellis. Device per step (raw bass, explicit
sems; every dep is a standalone wait_ge so the 1-sync-wait codegen limit
never binds):
    DVE: e_all = Ee[:, l, :, :] * v (broadcast over j)      [120, 3, 20]
    PE : psum  = B1^T @ e_all        (block-ones matmul -> s[b, j] on all i'')
    DVE: junk  = psum * D3 ; v[:, t] = sum_j junk           (diagonal pick)
Gold energy, final log/sum: O(L*B) host work on tiny tensors.
"""
import numpy as np

import concourse.bass as bass
import concourse.mybir as mybir
from concourse.bass_utils import run_bass_kernel_spmd

T = 20
START, END = 17, 18
KAPPA = 3.7881286
L, B = 512, 128
NCORES = 8
BS = B // NCORES
NEG = -100.0
F32 = mybir.dt.float32
CHUNK = 128
NCHUNK = L // CHUNK
GROUPS = [6, 6, 4]
PROFILE_DIR = None
LAST_RESULT = None
EBUFS = 3  # e_all ping-pong depth (lets one s_mm wait per step subsume WARs)


def _build_bass():
    nc = bass.Bass("TRN2", num_devices=NCORES, detect_race_conditions=False)
    A_d = nc.declare_dram_parameter("A", [120, L * 60], F32, isOutput=False)
    v0_d = nc.declare_dram_parameter("v0", [120, 3], F32, isOutput=False)
    D3_d = nc.declare_dram_parameter("D3", [120, 60], F32, isOutput=False)
    B1_d = nc.declare_dram_parameter("B1", [120, 120], F32, isOutput=False)
    out_d = nc.declare_dram_parameter("vout", [120, 3], F32, isOutput=True)

    add = mybir.AluOpType.add
    import contextlib

    es = contextlib.ExitStack()
    with es:
        Ee = [es.enter_context(nc.sbuf_tensor(f"Ee{i}", [120, CHUNK * 60], F32)) for i in range(2)]
        raw = [es.enter_context(nc.sbuf_tensor(f"raw{i}", [120, CHUNK * 60], F32)) for i in range(2)]
        e_all = es.enter_context(nc.sbuf_tensor("e_all", [120, EBUFS * 60], F32))
        junk = es.enter_context(nc.sbuf_tensor("junk", [120, 60], F32))
        v_sb = es.enter_context(nc.sbuf_tensor("v_sb", [120, 3], F32))
        D3_sb = es.enter_context(nc.sbuf_tensor("D3_sb", [120, 60], F32))
        B1_sb = es.enter_context(nc.sbuf_tensor("B1_sb", [120, 120], F32))
        psum = es.enter_context(nc.psum_tensor("ps", [120, 60], F32))

        s_cd = es.enter_context(nc.semaphore("s_cd"))
        s_dma = es.enter_context(nc.semaphore("s_dma"))
        s_exp = es.enter_context(nc.semaphore("s_exp"))
        s_e = es.enter_context(nc.semaphore("s_e"))
        s_mm = es.enter_context(nc.semaphore("s_mm"))
        s_v = es.enter_context(nc.semaphore("s_v"))
        block = es.enter_context(nc.Block())

        @block.sync
        def _(sync):
            sync.dma_start(B1_sb[:, :], B1_d[:, :]).then_inc(s_cd, 16)
            sync.dma_start(D3_sb[:, :], D3_d[:, :]).then_inc(s_cd, 16)
            sync.dma_start(v_sb[:, :], v0_d[:, :]).then_inc(s_cd, 16)
            for c in range(NCHUNK):
                if c >= 2:
                    sync.wait_ge(s_exp, c - 1)  # exp(c-2) done reading raw buf
                sync.dma_start(
                    raw[c % 2][:, :], A_d[:, c * CHUNK * 60 : (c + 1) * CHUNK * 60]
                ).then_inc(s_dma, 16)
            sync.wait_ge(s_v, L - 1)
            sync.dma_start(out_d[:, :], v_sb[:, :]).then_inc(s_dma, 16)

        @block.scalar
        def _(scalar):
            for c in range(NCHUNK):
                scalar.wait_ge(s_dma, 16 * (c + 1))
                if c >= 2:
                    # ts(last l of chunk c-2) has read Ee buf -> safe to rewrite
                    scalar.wait_ge(s_v, (c - 1) * CHUNK - 1)
                scalar.activation(
                    Ee[c % 2][:, :],
                    raw[c % 2][:, :],
                    mybir.ActivationFunctionType.Exp,
                ).then_inc(s_exp, 1)

        @block.tensor
        def _(tensor):
            tensor.wait_ge(s_cd, 48)
            for l in range(1, L):
                tensor.wait_ge(s_e, l)
                eb = l % EBUFS
                tensor.matmul(
                    psum[:, :],
                    B1_sb[:, :],
                    e_all[:, eb * 60 : (eb + 1) * 60],
                    start=True,
                    stop=True,
                ).then_inc(s_mm, 1)

        @block.vector
        def _(vector):
            vector.wait_ge(s_cd, 48)
            for l in range(1, L):
                c, lc = divmod(l, CHUNK)
                if lc == 0 or l == 1:
                    vector.wait_ge(s_exp, c + 1)
                eb = l % EBUFS
                ee = Ee[c % 2]
                src = bass.AP(ee, lc * 60, [[CHUNK * 60, 120], [20, 3], [1, 20]])
                dst = bass.AP(e_all, eb * 60, [[EBUFS * 60, 120], [20, 3], [1, 20]])
                vbc = bass.AP(v_sb, 0, [[3, 120], [1, 3], [0, 20]])
                vector.tensor_mul(dst, src, vbc).then_inc(s_e, 1)
                vector.wait_ge(s_mm, l)
                vector.tensor_mul(junk[:, :], psum[:, :], D3_sb[:, :])
                vector.tensor_reduce(
                    bass.AP(v_sb, 0, [[3, 120], [1, 3]]),
                    bass.AP(junk, 0, [[60, 120], [20, 3], [1, 20]]),
                    axis=mybir.AxisListType.X,
                    op=add,
                ).then_inc(s_v, 1)

    return nc


_NC_CACHE = None


def _get_nc():
    global _NC_CACHE
    if _NC_CACHE is None:
        _NC_CACHE = _build_bass()
    return _NC_CACHE


def kernel(trellis, gold, mask, corpus_mask):
    trellis = np.asarray(trellis, dtype=np.float32)
    gold = np.asarray(gold).astype(np.int64)
    mask = np.asarray(mask).astype(bool)

    # ---- host: gold energy (O(L*B) gather) + per-b unmasked counts ----
    flat = trellis.reshape(L, B, T * T)
    energy = np.take_along_axis(flat, gold, axis=2)[..., 0]
    gold_energy = float(np.sum(np.where(mask, energy.astype(np.float64), 0.0)))
    cnt = mask[1:].sum(axis=0).astype(np.float64)

    # ---- host: fold chem cost, mask->identity, -kappa into log trellis ----
    chem = np.zeros((T,), np.float32)
    chem[:4] = 1.0
    lnI = np.full((T, T), NEG, np.float32)
    np.fill_diagonal(lnI, 0.0)
    Tt = trellis + (chem - np.float32(KAPPA))[None, None, None, :]
    Tt = np.where(mask[:, :, None, None], Tt, lnI[None, None])
    Tt[0] = lnI[None]

    D3 = np.zeros((120, 60), np.float32)
    for p in range(120):
        for t in range(3):
            D3[p, t * T + p % T] = 1.0
    B1 = np.zeros((120, 120), np.float32)
    for k in range(120):
        B1[k, (k // T) * T : (k // T + 1) * T] = 1.0

    in_maps = []
    for cidx in range(NCORES):
        A_l = np.empty((120, L, 3, T), np.float32)
        for t in range(3):
            for gg in range(6):
                rows = slice(20 * gg, 20 * gg + 20)
                if gg < GROUPS[t]:
                    b_loc = (6 * t + gg) if t < 2 else (12 + gg)
                    bg = cidx * BS + b_loc
                    A_l[rows, :, t, :] = np.transpose(Tt[:, bg], (1, 0, 2))
                else:
                    A_l[rows, :, t, :] = lnI[:, None, :]
        A = np.ascontiguousarray(A_l.reshape(120, L * 60))

        v0 = np.zeros((120, 3), np.float32)
        for t in range(3):
            for gg in range(GROUPS[t]):
                b_loc = (6 * t + gg) if t < 2 else (12 + gg)
                bg = cidx * BS + b_loc
                v0[20 * gg : 20 * gg + 20, t] = np.exp(trellis[0, bg, START, :])
        in_maps.append({"A": A, "v0": v0, "D3": D3, "B1": B1})

    nc = _get_nc()
    global LAST_RESULT
    kw = dict(trace=True, tmpdir=PROFILE_DIR) if PROFILE_DIR else {}
    LAST_RESULT = run_bass_kernel_spmd(nc, in_maps, list(range(NCORES)), **kw)
    res = LAST_RESULT.results

    partition = 0.0
    for cidx in range(NCORES):
        vout = np.asarray(res[cidx]["vout"], dtype=np.float64)
        for t in range(3):
            for gg in range(GROUPS[t]):
                b_loc = (6 * t + gg) if t < 2 else (12 + gg)
                bg = cidx * BS + b_loc
                partition += np.log(vout[20 * gg + END, t]) + KAPPA * cnt[bg]
    return np.float32((partition - gold_energy) / B)



# revision 3
# speedup vs baseline: 3.5774x; 3.5774x over previous
"""CRF loss kernel for Trainium2 (8 NeuronCores, batch-sharded).

Forward algorithm in exp space, restructured from the 4-instr/step baseline:
  * meet-in-the-middle: fwd chain (steps 1..256) + bwd chain (511..257 + one
    identity pad), merged into the same instructions -> FD=120 per DVE op.
  * even/odd layout alternation: even step = DVE mul (v broadcast over 'to')
    + PE block-ones matmul (group-sum over partitions, result replicated
    along free); odd step = DVE mul directly against PSUM + free-axis
    reduce -> back to partition layout. 3 DVE + 1 MM per 2 steps per dir.
  * bf16 slabs/operands (single-pass PE matmul instead of fp32 LOW/HIGH
    emulation; halved HBM traffic and ACT exp time). PSUM accum stays fp32.

Per bundle m (covers fwd steps 2m+1,2m+2 and bwd steps 512-2m,511-2m):
    DVE: e_all = Ee[even slab m] * v (broadcast over 'a')     [120 x 120]
    PE : psum  = B1^T @ e_all   (block-ones -> per-group sums, replicated)
    DVE: e2    = Ee[odd slab m] * psum
    DVE: v     = reduce_add(e2, innermost 'a')                [120 x 6]
Gold energy + final log/dot: O(L*B) host work on tiny tensors.
"""
import numpy as np
import ml_dtypes

import concourse.bass as bass
import concourse.mybir as mybir
from concourse.bass_utils import run_bass_kernel_spmd

T = 20
START, END = 17, 18
KAPPA = 3.7881286
L, B = 512, 128
NCORES = 8
BS = B // NCORES
NEG = -100.0
F32 = mybir.dt.float32
BF16 = mybir.dt.bfloat16
NPBF16 = ml_dtypes.bfloat16
NB = 128          # bundles; each consumes 4 trellis steps (2 fwd + 2 bwd)
ROW = NB * 240    # free elems per partition of A: 240 per bundle (2 slabs x 120)
NCHUNK = 4
CB = NB // NCHUNK  # bundles per exp chunk
PROFILE_DIR = None
LAST_RESULT = None

# slot map: (g in 0..5, t in 0..2) -> local batch index; t=2 uses groups 0..3
SLOTS = {}
for _t in range(3):
    for _g in range(6):
        if _t == 2 and _g >= 4:
            continue
        SLOTS[(_g, _t)] = (6 * _t + _g) if _t < 2 else (12 + _g)

FE = [2 * m + 1 for m in range(NB)]               # fwd even steps
FO = [2 * m + 2 for m in range(NB)]               # fwd odd steps
BE = [None] + [512 - 2 * m for m in range(1, NB)]  # bwd even (None = pad I)
BO = [511 - 2 * m for m in range(NB)]             # bwd odd steps


def _build_bass():
    nc = bass.Bass("TRN2", num_devices=NCORES, detect_race_conditions=False)
    A_d = nc.declare_dram_parameter("A", [120, ROW], BF16, isOutput=False)
    v0_d = nc.declare_dram_parameter("v0", [120, 6], BF16, isOutput=False)
    B1_d = nc.declare_dram_parameter("B1", [120, 120], BF16, isOutput=False)
    out_d = nc.declare_dram_parameter("vout", [120, 6], BF16, isOutput=True)

    add = mybir.AluOpType.add
    import contextlib

    es = contextlib.ExitStack()
    with es:
        Araw = es.enter_context(nc.sbuf_tensor("Araw", [120, ROW], BF16))
        Ee = es.enter_context(nc.sbuf_tensor("Ee", [120, ROW], BF16))
        e_all = es.enter_context(nc.sbuf_tensor("e_all", [120, 120], BF16))
        e2 = es.enter_context(nc.sbuf_tensor("e2", [120, 120], BF16))
        v_sb = es.enter_context(nc.sbuf_tensor("v_sb", [120, 6], BF16))
        B1_sb = es.enter_context(nc.sbuf_tensor("B1_sb", [120, 120], BF16))
        psum = es.enter_context(nc.psum_tensor("ps", [120, 120], F32))

        s_cd = es.enter_context(nc.semaphore("s_cd"))
        s_dma = es.enter_context(nc.semaphore("s_dma"))
        s_exp = es.enter_context(nc.semaphore("s_exp"))
        s_e = es.enter_context(nc.semaphore("s_e"))
        s_mm = es.enter_context(nc.semaphore("s_mm"))
        s_om = es.enter_context(nc.semaphore("s_om"))
        s_v = es.enter_context(nc.semaphore("s_v"))
        block = es.enter_context(nc.Block())

        @block.sync
        def _(sync):
            sync.dma_start(B1_sb[:, :], B1_d[:, :]).then_inc(s_cd, 16)
            sync.dma_start(v_sb[:, :], v0_d[:, :]).then_inc(s_cd, 16)
            for c in range(NCHUNK):
                sync.dma_start(
                    Araw[:, c * CB * 240 : (c + 1) * CB * 240],
                    A_d[:, c * CB * 240 : (c + 1) * CB * 240],
                ).then_inc(s_dma, 16)
            sync.wait_ge(s_v, NB)
            sync.dma_start(out_d[:, :], v_sb[:, :]).then_inc(s_dma, 16)

        @block.scalar
        def _(scalar):
            for c in range(NCHUNK):
                scalar.wait_ge(s_dma, 16 * (c + 1))
                scalar.activation(
                    Ee[:, c * CB * 240 : (c + 1) * CB * 240],
                    Araw[:, c * CB * 240 : (c + 1) * CB * 240],
                    mybir.ActivationFunctionType.Exp,
                ).then_inc(s_exp, 1)

        @block.tensor
        def _(tensor):
            tensor.wait_ge(s_cd, 32)
            for m in range(NB):
                tensor.wait_ge(s_e, m + 1)
                if m > 0:
                    tensor.wait_ge(s_om, m)  # psum consumed by odd-mul m-1
                tensor.matmul(
                    psum[:, :], B1_sb[:, :], e_all[:, :], start=True, stop=True
                ).then_inc(s_mm, 1)

        @block.vector
        def _(vector):
            vector.wait_ge(s_cd, 32)
            vbc = bass.AP(v_sb, 0, [[6, 120], [1, 6], [0, 20]])
            red_out = bass.AP(v_sb, 0, [[6, 120], [1, 6]])
            red_in = bass.AP(e2, 0, [[120, 120], [20, 6], [1, 20]])
            for m in range(NB):
                if m % CB == 0:
                    vector.wait_ge(s_exp, m // CB + 1)
                ev = bass.AP(Ee, m * 240, [[ROW, 120], [20, 6], [1, 20]])
                od = bass.AP(Ee, m * 240 + 120, [[ROW, 120], [20, 6], [1, 20]])
                vector.tensor_mul(e_all[:, :], ev, vbc).then_inc(s_e, 1)
                vector.wait_ge(s_mm, m + 1)
                vector.tensor_mul(e2[:, :], od, psum[:, :]).then_inc(s_om, 1)
                with nc.allow_low_precision(
                    reason="bf16 state validated: loss rel err ~1e-5 vs 2e-2 gate"
                ):
                    vector.tensor_reduce(
                        red_out, red_in, axis=mybir.AxisListType.X, op=add
                    ).then_inc(s_v, 1)

    return nc


_NC_CACHE = None


def _get_nc():
    global _NC_CACHE
    if _NC_CACHE is None:
        _NC_CACHE = _build_bass()
    return _NC_CACHE


def kernel(trellis, gold, mask, corpus_mask):
    trellis = np.asarray(trellis, dtype=np.float32)
    gold = np.asarray(gold).astype(np.int64)
    mask = np.asarray(mask).astype(bool)

    # ---- host: gold energy (O(L*B) gather) + per-b unmasked counts ----
    flat = trellis.reshape(L, B, T * T)
    energy = np.take_along_axis(flat, gold, axis=2)[..., 0]
    gold_energy = float(np.sum(np.where(mask, energy.astype(np.float64), 0.0)))
    cnt = mask[1:].sum(axis=0).astype(np.float64)

    # ---- host: fold chem cost, mask->identity, -kappa into log trellis ----
    chem = np.zeros((T,), np.float32)
    chem[:4] = 1.0
    lnI = np.full((T, T), np.float32(NEG))
    np.fill_diagonal(lnI, 0.0)
    Tt = trellis + (chem - np.float32(KAPPA))[None, None, None, :]
    Tt = np.where(mask[:, :, None, None], Tt, lnI[None, None])

    B1 = np.zeros((120, 120), NPBF16)
    for k in range(120):
        B1[k, (k // T) * T : (k // T + 1) * T] = 1.0

    be_real = [512 - 2 * m for m in range(1, NB)]  # BE without the m=0 pad

    in_maps = []
    for cidx in range(NCORES):
        A6 = np.empty((120, NB, 2, 2, 3, 20), np.float32)
        A6[:] = lnI[0, 1]  # scratch fill; every slot overwritten below
        for (g, t), b_loc in SLOTS.items():
            bg = cidx * BS + b_loc
            rows = slice(20 * g, 20 * g + 20)
            A6[rows, :, 0, 0, t, :] = Tt[FE, bg].transpose(1, 0, 2)
            A6[rows, 0, 0, 1, t, :] = lnI
            A6[rows, 1:, 0, 1, t, :] = Tt[be_real, bg].transpose(2, 0, 1)
            A6[rows, :, 1, 0, t, :] = Tt[FO, bg].transpose(2, 0, 1)
            A6[rows, :, 1, 1, t, :] = Tt[BO, bg].transpose(1, 0, 2)
        for t in range(3):
            for g in range(6):
                if (g, t) in SLOTS:
                    continue
                A6[20 * g : 20 * g + 20, :, :, :, t, :] = lnI[:, None, None, None, :]
        A = np.ascontiguousarray(A6.reshape(120, ROW)).astype(NPBF16)

        v0 = np.zeros((120, 2, 3), np.float32)
        for (g, t), b_loc in SLOTS.items():
            bg = cidx * BS + b_loc
            v0[20 * g : 20 * g + 20, 0, t] = np.exp(trellis[0, bg, START, :])
            v0[20 * g + END, 1, t] = 1.0
        in_maps.append(
            {"A": A, "v0": v0.reshape(120, 6).astype(NPBF16), "B1": B1}
        )

    nc = _get_nc()
    global LAST_RESULT
    kw = dict(trace=True, tmpdir=PROFILE_DIR) if PROFILE_DIR else {}
    LAST_RESULT = run_bass_kernel_spmd(nc, in_maps, list(range(NCORES)), **kw)
    res = LAST_RESULT.results

    partition = 0.0
    for cidx in range(NCORES):
        vout = np.asarray(res[cidx]["vout"], dtype=np.float64).reshape(120, 2, 3)
        for (g, t), b_loc in SLOTS.items():
            bg = cidx * BS + b_loc
            rows = slice(20 * g, 20 * g + 20)
            Z = float((vout[rows, 0, t] * vout[rows, 1, t]).sum())
            partition += np.log(Z) + KAPPA * cnt[bg]
    return np.float32((partition - gold_energy) / B)


# revision 4
# speedup vs baseline: 4.2053x; 1.1755x over previous
"""CRF loss kernel for Trainium2 (8 NeuronCores, batch-sharded).

Forward algorithm in exp space, v2 "skewed two-chain pipeline":
  * meet-in-the-middle: fwd chain (steps 1..256) + bwd chain (511..257 + one
    identity pad). The two chains run as separate FD=60 instruction streams,
    phase-shifted by half a round so each chain's PE matmul (group-sum)
    latency is hidden behind the other chain's DVE ops.
  * even/odd layout alternation per chain: even step = DVE mul (state
    broadcast over 'a') + PE block-ones matmul (partition group-sum,
    replicated along free); odd step = DVE mul against PSUM + free-axis
    reduce back to partition layout.
  * bf16 operands everywhere (fp32 PSUM accumulate); B1 padded to 128
    weight columns (fast weight load).

DVE round r:  em_f(r) | om_b(r-1) red_b(r-1) | em_b(r) | om_f(r) red_f(r)
PE  round r:  MM_f(r) (after em_f) , MM_b(r) (after em_b)
Gold energy + final log/dot: O(L*B) host work on tiny tensors.
"""
import numpy as np
import ml_dtypes

import concourse.bass as bass
import concourse.mybir as mybir
from concourse.bass_utils import run_bass_kernel_spmd

T = 20
START, END = 17, 18
KAPPA = 3.7881286
L, B = 512, 128
NCORES = 8
BS = B // NCORES
NEG = -100.0
F32 = mybir.dt.float32
BF16 = mybir.dt.bfloat16
NPBF16 = ml_dtypes.bfloat16
NB = 128          # rounds; each consumes 4 trellis steps (2 fwd + 2 bwd)
ROW = NB * 240    # free elems per partition of A: 240 per round
NCHUNK = 4
CB = NB // NCHUNK  # rounds per exp chunk
PROFILE_DIR = None
LAST_RESULT = None

# slot map: (g in 0..5, t in 0..2) -> local batch index; t=2 uses groups 0..3
SLOTS = {}
for _t in range(3):
    for _g in range(6):
        if _t == 2 and _g >= 4:
            continue
        SLOTS[(_g, _t)] = (6 * _t + _g) if _t < 2 else (12 + _g)

FE = [2 * m + 1 for m in range(NB)]               # fwd even steps
FO = [2 * m + 2 for m in range(NB)]               # fwd odd steps
BE = [None] + [512 - 2 * m for m in range(1, NB)]  # bwd even (None = pad I)
BO = [511 - 2 * m for m in range(NB)]             # bwd odd steps


def _build_bass():
    nc = bass.Bass("TRN2", num_devices=NCORES, detect_race_conditions=False)
    A_d = nc.declare_dram_parameter("A", [120, ROW], BF16, isOutput=False)
    v0f_d = nc.declare_dram_parameter("v0f", [120, 3], BF16, isOutput=False)
    v0b_d = nc.declare_dram_parameter("v0b", [120, 3], BF16, isOutput=False)
    B1_d = nc.declare_dram_parameter("B1", [120, 128], BF16, isOutput=False)
    outf_d = nc.declare_dram_parameter("voutf", [120, 3], BF16, isOutput=True)
    outb_d = nc.declare_dram_parameter("voutb", [120, 3], BF16, isOutput=True)

    add = mybir.AluOpType.add
    X = mybir.AxisListType.X
    import contextlib

    es = contextlib.ExitStack()
    with es:
        Araw = es.enter_context(nc.sbuf_tensor("Araw", [120, ROW], BF16))
        Ee = es.enter_context(nc.sbuf_tensor("Ee", [120, ROW], BF16))
        ea_f = es.enter_context(nc.sbuf_tensor("ea_f", [120, 60], BF16))
        ea_b = es.enter_context(nc.sbuf_tensor("ea_b", [120, 60], BF16))
        e2_f = es.enter_context(nc.sbuf_tensor("e2_f", [120, 60], BF16))
        e2_b = es.enter_context(nc.sbuf_tensor("e2_b", [120, 60], BF16))
        v_f = es.enter_context(nc.sbuf_tensor("v_f", [120, 3], BF16))
        v_b = es.enter_context(nc.sbuf_tensor("v_b", [120, 3], BF16))
        B1_sb = es.enter_context(nc.sbuf_tensor("B1_sb", [120, 128], BF16))
        ps_f = es.enter_context(nc.psum_tensor("ps_f", [128, 60], F32))
        ps_b = es.enter_context(nc.psum_tensor("ps_b", [128, 60], F32))

        s_cd = es.enter_context(nc.semaphore("s_cd"))
        s_dma = es.enter_context(nc.semaphore("s_dma"))
        s_exp = es.enter_context(nc.semaphore("s_exp"))
        s_ef = es.enter_context(nc.semaphore("s_ef"))
        s_eb = es.enter_context(nc.semaphore("s_eb"))
        s_mmf = es.enter_context(nc.semaphore("s_mmf"))
        s_mmb = es.enter_context(nc.semaphore("s_mmb"))
        s_omf = es.enter_context(nc.semaphore("s_omf"))
        s_omb = es.enter_context(nc.semaphore("s_omb"))
        s_vf = es.enter_context(nc.semaphore("s_vf"))
        s_vb = es.enter_context(nc.semaphore("s_vb"))
        block = es.enter_context(nc.Block())

        @block.sync
        def _(sync):
            sync.dma_start(B1_sb[:, :], B1_d[:, :]).then_inc(s_cd, 16)
            sync.dma_start(v_f[:, :], v0f_d[:, :]).then_inc(s_cd, 16)
            sync.dma_start(v_b[:, :], v0b_d[:, :]).then_inc(s_cd, 16)
            for c in range(NCHUNK):
                sync.dma_start(
                    Araw[:, c * CB * 240 : (c + 1) * CB * 240],
                    A_d[:, c * CB * 240 : (c + 1) * CB * 240],
                ).then_inc(s_dma, 16)
            sync.wait_ge(s_vf, NB)
            sync.wait_ge(s_vb, NB)
            sync.dma_start(outf_d[:, :], v_f[:, :]).then_inc(s_dma, 16)
            sync.dma_start(outb_d[:, :], v_b[:, :]).then_inc(s_dma, 16)

        @block.scalar
        def _(scalar):
            for c in range(NCHUNK):
                scalar.wait_ge(s_dma, 16 * (c + 1))
                scalar.activation(
                    Ee[:, c * CB * 240 : (c + 1) * CB * 240],
                    Araw[:, c * CB * 240 : (c + 1) * CB * 240],
                    mybir.ActivationFunctionType.Exp,
                ).then_inc(s_exp, 1)

        @block.tensor
        def _(tensor):
            tensor.wait_ge(s_cd, 48)
            for r in range(NB):
                tensor.wait_ge(s_ef, r + 1)
                if r > 0:
                    tensor.wait_ge(s_omf, r)   # ps_f consumed by om_f(r-1)
                tensor.matmul(
                    ps_f[:, :], B1_sb[:, :], ea_f[:, :], start=True, stop=True
                ).then_inc(s_mmf, 1)
                tensor.wait_ge(s_eb, r + 1)
                if r > 0:
                    tensor.wait_ge(s_omb, r)   # ps_b consumed by om_b(r-1)
                tensor.matmul(
                    ps_b[:, :], B1_sb[:, :], ea_b[:, :], start=True, stop=True
                ).then_inc(s_mmb, 1)

        @block.vector
        def _(vector):
            vector.wait_ge(s_cd, 48)
            vbcf = bass.AP(v_f, 0, [[3, 120], [1, 3], [0, 20]])
            vbcb = bass.AP(v_b, 0, [[3, 120], [1, 3], [0, 20]])
            rof = bass.AP(v_f, 0, [[3, 120], [1, 3]])
            rob = bass.AP(v_b, 0, [[3, 120], [1, 3]])
            rif = bass.AP(e2_f, 0, [[60, 120], [20, 3], [1, 20]])
            rib = bass.AP(e2_b, 0, [[60, 120], [20, 3], [1, 20]])
            psf120 = bass.AP(ps_f, 0, [[60, 120], [1, 60]])
            psb120 = bass.AP(ps_b, 0, [[60, 120], [1, 60]])

            def ap(r, eo, s):
                return bass.AP(
                    Ee, r * 240 + (eo * 2 + s) * 60, [[ROW, 120], [20, 3], [1, 20]]
                )

            lp = nc.allow_low_precision(
                reason="bf16 state validated: loss rel err ~1e-4 vs 2e-2 gate"
            )
            with lp:
                for r in range(NB):
                    if r % CB == 0:
                        vector.wait_ge(s_exp, r // CB + 1)
                    vector.tensor_mul(ea_f[:, :], ap(r, 0, 0), vbcf).then_inc(s_ef, 1)
                    if r > 0:
                        vector.wait_ge(s_mmb, r)
                        vector.tensor_mul(e2_b[:, :], ap(r - 1, 1, 1), psb120).then_inc(s_omb, 1)
                        vector.tensor_reduce(rob, rib, axis=X, op=add).then_inc(s_vb, 1)
                    vector.tensor_mul(ea_b[:, :], ap(r, 0, 1), vbcb).then_inc(s_eb, 1)
                    vector.wait_ge(s_mmf, r + 1)
                    vector.tensor_mul(e2_f[:, :], ap(r, 1, 0), psf120).then_inc(s_omf, 1)
                    vector.tensor_reduce(rof, rif, axis=X, op=add).then_inc(s_vf, 1)
                # drain: last bwd odd step
                vector.wait_ge(s_mmb, NB)
                vector.tensor_mul(e2_b[:, :], ap(NB - 1, 1, 1), psb120).then_inc(s_omb, 1)
                vector.tensor_reduce(rob, rib, axis=X, op=add).then_inc(s_vb, 1)

    return nc


_NC_CACHE = None


def _get_nc():
    global _NC_CACHE
    if _NC_CACHE is None:
        _NC_CACHE = _build_bass()
    return _NC_CACHE


def kernel(trellis, gold, mask, corpus_mask):
    trellis = np.asarray(trellis, dtype=np.float32)
    gold = np.asarray(gold).astype(np.int64)
    mask = np.asarray(mask).astype(bool)

    # ---- host: gold energy (O(L*B) gather) + per-b unmasked counts ----
    flat = trellis.reshape(L, B, T * T)
    energy = np.take_along_axis(flat, gold, axis=2)[..., 0]
    gold_energy = float(np.sum(np.where(mask, energy.astype(np.float64), 0.0)))
    cnt = mask[1:].sum(axis=0).astype(np.float64)

    # ---- host: fold chem cost, mask->identity, -kappa into log trellis ----
    chem = np.zeros((T,), np.float32)
    chem[:4] = 1.0
    lnI = np.full((T, T), np.float32(NEG))
    np.fill_diagonal(lnI, 0.0)
    Tt = trellis + (chem - np.float32(KAPPA))[None, None, None, :]
    Tt = np.where(mask[:, :, None, None], Tt, lnI[None, None])

    B1 = np.zeros((120, 128), NPBF16)
    for k in range(120):
        B1[k, (k // T) * T : (k // T + 1) * T] = 1.0

    be_real = [512 - 2 * m for m in range(1, NB)]  # BE without the m=0 pad

    in_maps = []
    for cidx in range(NCORES):
        A6 = np.empty((120, NB, 2, 2, 3, 20), np.float32)
        for (g, t), b_loc in SLOTS.items():
            bg = cidx * BS + b_loc
            rows = slice(20 * g, 20 * g + 20)
            A6[rows, :, 0, 0, t, :] = Tt[FE, bg].transpose(1, 0, 2)
            A6[rows, 0, 0, 1, t, :] = lnI
            A6[rows, 1:, 0, 1, t, :] = Tt[be_real, bg].transpose(2, 0, 1)
            A6[rows, :, 1, 0, t, :] = Tt[FO, bg].transpose(2, 0, 1)
            A6[rows, :, 1, 1, t, :] = Tt[BO, bg].transpose(1, 0, 2)
        for t in range(3):
            for g in range(6):
                if (g, t) in SLOTS:
                    continue
                A6[20 * g : 20 * g + 20, :, :, :, t, :] = lnI[:, None, None, None, :]
        A = np.ascontiguousarray(A6.reshape(120, ROW)).astype(NPBF16)

        v0 = np.zeros((120, 2, 3), np.float32)
        for (g, t), b_loc in SLOTS.items():
            bg = cidx * BS + b_loc
            v0[20 * g : 20 * g + 20, 0, t] = np.exp(trellis[0, bg, START, :])
            v0[20 * g + END, 1, t] = 1.0
        in_maps.append(
            {
                "A": A,
                "v0f": v0[:, 0].astype(NPBF16),
                "v0b": v0[:, 1].astype(NPBF16),
                "B1": B1,
            }
        )

    nc = _get_nc()
    global LAST_RESULT
    kw = dict(trace=True, tmpdir=PROFILE_DIR) if PROFILE_DIR else {}
    LAST_RESULT = run_bass_kernel_spmd(nc, in_maps, list(range(NCORES)), **kw)
    res = LAST_RESULT.results

    partition = 0.0
    for cidx in range(NCORES):
        vf = np.asarray(res[cidx]["voutf"], dtype=np.float64)
        vb = np.asarray(res[cidx]["voutb"], dtype=np.float64)
        for (g, t), b_loc in SLOTS.items():
            bg = cidx * BS + b_loc
            rows = slice(20 * g, 20 * g + 20)
            Z = float((vf[rows, t] * vb[rows, t]).sum())
            partition += np.log(Z) + KAPPA * cnt[bg]
    return np.float32((partition - gold_energy) / B)


# revision 9
# speedup vs baseline: 4.8778x; 1.1599x over previous
"""CRF loss kernel for Trainium2 (8 NeuronCores, batch-sharded).

Forward algorithm in exp space, v2 "skewed two-chain pipeline":
  * meet-in-the-middle: fwd chain (steps 1..256) + bwd chain (511..257 + one
    identity pad). The two chains run as separate FD=60 instruction streams,
    phase-shifted by half a round so each chain's PE matmul (group-sum)
    latency is hidden behind the other chain's DVE ops.
  * even/odd layout alternation per chain: even step = DVE mul (state
    broadcast over 'a') + PE block-ones matmul (partition group-sum,
    replicated along free); odd step = DVE mul against PSUM + free-axis
    reduce back to partition layout.
  * bf16 operands everywhere (fp32 PSUM accumulate); B1 padded to 128
    weight columns (fast weight load).

DVE round r:  em_f(r) | om_b(r-1) red_b(r-1) | em_b(r) | om_f(r) red_f(r)
PE  round r:  MM_f(r) (after em_f) , MM_b(r) (after em_b)
Gold energy + final log/dot: O(L*B) host work on tiny tensors.
"""
import numpy as np
import ml_dtypes

import concourse.bass as bass
import concourse.mybir as mybir
from concourse.bass_utils import run_bass_kernel_spmd

T = 20
START, END = 17, 18
KAPPA = 3.7881286
L, B = 512, 128
NCORES = 8
BS = B // NCORES
NEG = -100.0
F32 = mybir.dt.float32
BF16 = mybir.dt.bfloat16
NPBF16 = ml_dtypes.bfloat16
NB = 128          # rounds; each consumes 4 trellis steps (2 fwd + 2 bwd)
ROW = NB * 240    # free elems per partition of A: 240 per round
NCHUNK = 8
CB = NB // NCHUNK  # rounds per exp chunk
PROFILE_DIR = None
LAST_RESULT = None

# slot map: (g in 0..5, t in 0..2) -> local batch index; t=2 uses groups 0..3
SLOTS = {}
for _t in range(3):
    for _g in range(6):
        if _t == 2 and _g >= 4:
            continue
        SLOTS[(_g, _t)] = (6 * _t + _g) if _t < 2 else (12 + _g)

FE = [2 * m + 1 for m in range(NB)]               # fwd even steps
FO = [2 * m + 2 for m in range(NB)]               # fwd odd steps
BE = [None] + [512 - 2 * m for m in range(1, NB)]  # bwd even (None = pad I)
BO = [511 - 2 * m for m in range(NB)]             # bwd odd steps


def _build_bass():
    nc = bass.Bass("TRN2", num_devices=NCORES, detect_race_conditions=False)
    A_d = nc.declare_dram_parameter("A", [120, ROW], BF16, isOutput=False)
    v0f_d = nc.declare_dram_parameter("v0f", [120, 3], BF16, isOutput=False)
    v0b_d = nc.declare_dram_parameter("v0b", [120, 3], BF16, isOutput=False)
    B1_d = nc.declare_dram_parameter("B1", [120, 128], BF16, isOutput=False)
    outf_d = nc.declare_dram_parameter("voutf", [120, 3], BF16, isOutput=True)
    outb_d = nc.declare_dram_parameter("voutb", [120, 3], BF16, isOutput=True)

    add = mybir.AluOpType.add
    X = mybir.AxisListType.X
    import contextlib

    es = contextlib.ExitStack()
    with es:
        Araw = es.enter_context(nc.sbuf_tensor("Araw", [120, ROW], BF16))
        Ee = es.enter_context(nc.sbuf_tensor("Ee", [120, ROW], BF16))
        ea_f = es.enter_context(nc.sbuf_tensor("ea_f", [120, 60], BF16))
        ea_b = es.enter_context(nc.sbuf_tensor("ea_b", [120, 60], BF16))
        e2_f = es.enter_context(nc.sbuf_tensor("e2_f", [120, 60], BF16))
        e2_b = es.enter_context(nc.sbuf_tensor("e2_b", [120, 60], BF16))
        v_f = es.enter_context(nc.sbuf_tensor("v_f", [120, 3], BF16))
        v_b = es.enter_context(nc.sbuf_tensor("v_b", [120, 3], BF16))
        B1_sb = es.enter_context(nc.sbuf_tensor("B1_sb", [120, 128], BF16))
        ps_f = [
            es.enter_context(nc.psum_tensor(f"ps_f{i}", [128, 60], F32))
            for i in range(2)
        ]
        ps_b = [
            es.enter_context(nc.psum_tensor(f"ps_b{i}", [128, 60], F32))
            for i in range(2)
        ]

        s_cd = es.enter_context(nc.semaphore("s_cd"))
        s_dma = es.enter_context(nc.semaphore("s_dma"))
        s_exp = es.enter_context(nc.semaphore("s_exp"))
        s_ef = es.enter_context(nc.semaphore("s_ef"))
        s_eb = es.enter_context(nc.semaphore("s_eb"))
        s_mmf = es.enter_context(nc.semaphore("s_mmf"))
        s_mmb = es.enter_context(nc.semaphore("s_mmb"))
        s_vf = es.enter_context(nc.semaphore("s_vf"))
        s_vb = es.enter_context(nc.semaphore("s_vb"))
        block = es.enter_context(nc.Block())

        @block.sync
        def _(sync):
            sync.dma_start(B1_sb[:, :], B1_d[:, :]).then_inc(s_cd, 16)
            sync.dma_start(v_f[:, :], v0f_d[:, :]).then_inc(s_cd, 16)
            sync.dma_start(v_b[:, :], v0b_d[:, :]).then_inc(s_cd, 16)
            for c in range(NCHUNK):
                sync.dma_start(
                    Araw[:, c * CB * 240 : (c + 1) * CB * 240],
                    A_d[:, c * CB * 240 : (c + 1) * CB * 240],
                ).then_inc(s_dma, 16)
            sync.wait_ge(s_vf, NB)
            sync.wait_ge(s_vb, NB)
            sync.dma_start(outf_d[:, :], v_f[:, :]).then_inc(s_dma, 16)
            sync.dma_start(outb_d[:, :], v_b[:, :]).then_inc(s_dma, 16)

        @block.scalar
        def _(scalar):
            for c in range(NCHUNK):
                scalar.wait_ge(s_dma, 16 * (c + 1))
                scalar.activation(
                    Ee[:, c * CB * 240 : (c + 1) * CB * 240],
                    Araw[:, c * CB * 240 : (c + 1) * CB * 240],
                    mybir.ActivationFunctionType.Exp,
                ).then_inc(s_exp, 1)

        @block.tensor
        def _(tensor):
            tensor.wait_ge(s_cd, 48)
            for r in range(NB):
                # psum double-buffered: bank r%2 was last read by om(r-2),
                # which precedes em(r) in DVE program order -> no WAR wait.
                tensor.wait_ge(s_ef, r + 1)
                tensor.matmul(
                    ps_f[r % 2][:, :], B1_sb[:, :], ea_f[:, :], start=True, stop=True
                ).then_inc(s_mmf, 1)
                tensor.wait_ge(s_eb, r + 1)
                tensor.matmul(
                    ps_b[r % 2][:, :], B1_sb[:, :], ea_b[:, :], start=True, stop=True
                ).then_inc(s_mmb, 1)

        @block.vector
        def _(vector):
            vector.wait_ge(s_cd, 48)
            vbcf = bass.AP(v_f, 0, [[3, 120], [1, 3], [0, 20]])
            vbcb = bass.AP(v_b, 0, [[3, 120], [1, 3], [0, 20]])
            rof = bass.AP(v_f, 0, [[3, 120], [1, 3]])
            rob = bass.AP(v_b, 0, [[3, 120], [1, 3]])
            rif = bass.AP(e2_f, 0, [[60, 120], [20, 3], [1, 20]])
            rib = bass.AP(e2_b, 0, [[60, 120], [20, 3], [1, 20]])
            psf120 = [bass.AP(p, 0, [[60, 120], [1, 60]]) for p in ps_f]
            psb120 = [bass.AP(p, 0, [[60, 120], [1, 60]]) for p in ps_b]

            def ap(r, eo, s):
                return bass.AP(
                    Ee, r * 240 + (eo * 2 + s) * 60, [[ROW, 120], [20, 3], [1, 20]]
                )

            lp = nc.allow_low_precision(
                reason="bf16 state validated: loss rel err ~1e-4 vs 2e-2 gate"
            )
            with lp:
                for r in range(NB):
                    if r % CB == 0:
                        vector.wait_ge(s_exp, r // CB + 1)
                    vector.tensor_mul(ea_f[:, :], ap(r, 0, 0), vbcf).then_inc(s_ef, 1)
                    if r > 0:
                        vector.wait_ge(s_mmb, r)
                        vector.tensor_mul(e2_b[:, :], ap(r - 1, 1, 1), psb120[(r - 1) % 2])
                        vector.tensor_reduce(rob, rib, axis=X, op=add).then_inc(s_vb, 1)
                    vector.tensor_mul(ea_b[:, :], ap(r, 0, 1), vbcb).then_inc(s_eb, 1)
                    vector.wait_ge(s_mmf, r + 1)
                    vector.tensor_mul(e2_f[:, :], ap(r, 1, 0), psf120[r % 2])
                    vector.tensor_reduce(rof, rif, axis=X, op=add).then_inc(s_vf, 1)
                # drain: last bwd odd step
                vector.wait_ge(s_mmb, NB)
                vector.tensor_mul(e2_b[:, :], ap(NB - 1, 1, 1), psb120[(NB - 1) % 2])
                vector.tensor_reduce(rob, rib, axis=X, op=add).then_inc(s_vb, 1)

    return nc


_NC_CACHE = None


def _get_nc():
    global _NC_CACHE
    if _NC_CACHE is None:
        _NC_CACHE = _build_bass()
    return _NC_CACHE


def kernel(trellis, gold, mask, corpus_mask):
    trellis = np.asarray(trellis, dtype=np.float32)
    gold = np.asarray(gold).astype(np.int64)
    mask = np.asarray(mask).astype(bool)

    # ---- host: gold energy (O(L*B) gather) + per-b unmasked counts ----
    flat = trellis.reshape(L, B, T * T)
    energy = np.take_along_axis(flat, gold, axis=2)[..., 0]
    gold_energy = float(np.sum(np.where(mask, energy.astype(np.float64), 0.0)))
    cnt = mask[1:].sum(axis=0).astype(np.float64)

    # ---- host: fold chem cost, mask->identity, -kappa into log trellis ----
    chem = np.zeros((T,), np.float32)
    chem[:4] = 1.0
    lnI = np.full((T, T), np.float32(NEG))
    np.fill_diagonal(lnI, 0.0)
    Tt = trellis + (chem - np.float32(KAPPA))[None, None, None, :]
    Tt = np.where(mask[:, :, None, None], Tt, lnI[None, None])

    B1 = np.zeros((120, 128), NPBF16)
    for k in range(120):
        B1[k, (k // T) * T : (k // T + 1) * T] = 1.0

    be_real = [512 - 2 * m for m in range(1, NB)]  # BE without the m=0 pad

    in_maps = []
    for cidx in range(NCORES):
        A6 = np.empty((120, NB, 2, 2, 3, 20), np.float32)
        for (g, t), b_loc in SLOTS.items():
            bg = cidx * BS + b_loc
            rows = slice(20 * g, 20 * g + 20)
            A6[rows, :, 0, 0, t, :] = Tt[FE, bg].transpose(1, 0, 2)
            A6[rows, 0, 0, 1, t, :] = lnI
            A6[rows, 1:, 0, 1, t, :] = Tt[be_real, bg].transpose(2, 0, 1)
            A6[rows, :, 1, 0, t, :] = Tt[FO, bg].transpose(2, 0, 1)
            A6[rows, :, 1, 1, t, :] = Tt[BO, bg].transpose(1, 0, 2)
        for t in range(3):
            for g in range(6):
                if (g, t) in SLOTS:
                    continue
                A6[20 * g : 20 * g + 20, :, :, :, t, :] = lnI[:, None, None, None, :]
        A = np.ascontiguousarray(A6.reshape(120, ROW)).astype(NPBF16)

        v0 = np.zeros((120, 2, 3), np.float32)
        for (g, t), b_loc in SLOTS.items():
            bg = cidx * BS + b_loc
            v0[20 * g : 20 * g + 20, 0, t] = np.exp(trellis[0, bg, START, :])
            v0[20 * g + END, 1, t] = 1.0
        in_maps.append(
            {
                "A": A,
                "v0f": v0[:, 0].astype(NPBF16),
                "v0b": v0[:, 1].astype(NPBF16),
                "B1": B1,
            }
        )

    nc = _get_nc()
    global LAST_RESULT
    kw = dict(trace=True, tmpdir=PROFILE_DIR) if PROFILE_DIR else {}
    LAST_RESULT = run_bass_kernel_spmd(nc, in_maps, list(range(NCORES)), **kw)
    res = LAST_RESULT.results

    partition = 0.0
    for cidx in range(NCORES):
        vf = np.asarray(res[cidx]["voutf"], dtype=np.float64)
        vb = np.asarray(res[cidx]["voutb"], dtype=np.float64)
        for (g, t), b_loc in SLOTS.items():
            bg = cidx * BS + b_loc
            rows = slice(20 * g, 20 * g + 20)
            Z = float((vf[rows, t] * vb[rows, t]).sum())
            partition += np.log(Z) + KAPPA * cnt[bg]
    return np.float32((partition - gold_energy) / B)


# revision 14
# speedup vs baseline: 5.0592x; 1.0372x over previous
"""CRF loss kernel for Trainium2 (8 NeuronCores, batch-sharded).

Forward algorithm in exp space, v2 "skewed two-chain pipeline":
  * meet-in-the-middle: fwd chain (steps 1..256) + bwd chain (511..257 + one
    identity pad). The two chains run as separate FD=60 instruction streams,
    phase-shifted by half a round so each chain's PE matmul (group-sum)
    latency is hidden behind the other chain's DVE ops.
  * even/odd layout alternation per chain: even step = DVE mul (state
    broadcast over 'a') + PE block-ones matmul (partition group-sum,
    replicated along free); odd step = DVE mul against PSUM + free-axis
    reduce back to partition layout.
  * bf16 operands everywhere (fp32 PSUM accumulate); B1 padded to 128
    weight columns (fast weight load).

DVE round r:  em_f(r) | om_b(r-1) red_b(r-1) | em_b(r) | om_f(r) red_f(r)
PE  round r:  MM_f(r) (after em_f) , MM_b(r) (after em_b)
Gold energy + final log/dot: O(L*B) host work on tiny tensors.
"""
import numpy as np
import ml_dtypes

import concourse.bass as bass
import concourse.mybir as mybir
from concourse.bass_utils import run_bass_kernel_spmd

T = 20
START, END = 17, 18
KAPPA = 3.7881286
L, B = 512, 128
NCORES = 8
BS = B // NCORES
NEG = -100.0
F32 = mybir.dt.float32
BF16 = mybir.dt.bfloat16
NPBF16 = ml_dtypes.bfloat16
NB = 128          # rounds; each consumes 4 trellis steps (2 fwd + 2 bwd)
ROW = NB * 240    # free elems per partition of A: 240 per round
# geometric chunk ramp (rounds per chunk): small first chunk so the first
# exp + first compute round start ASAP; later chunks amortize dispatch cost
CHUNKS = [4, 4, 8, 8, 16, 16, 24, 24, 24]
assert sum(CHUNKS) == NB
CHUNK_START = [sum(CHUNKS[:i]) for i in range(len(CHUNKS))]
# round r -> index of chunk containing it
R2C = []
for _ci, _n in enumerate(CHUNKS):
    R2C += [_ci] * _n
PROFILE_DIR = None
LAST_RESULT = None

# slot map: (g in 0..5, t in 0..2) -> local batch index; t=2 uses groups 0..3
SLOTS = {}
for _t in range(3):
    for _g in range(6):
        if _t == 2 and _g >= 4:
            continue
        SLOTS[(_g, _t)] = (6 * _t + _g) if _t < 2 else (12 + _g)

FE = [2 * m + 1 for m in range(NB)]               # fwd even steps
FO = [2 * m + 2 for m in range(NB)]               # fwd odd steps
BE = [None] + [512 - 2 * m for m in range(1, NB)]  # bwd even (None = pad I)
BO = [511 - 2 * m for m in range(NB)]             # bwd odd steps


def _build_bass():
    nc = bass.Bass("TRN2", num_devices=NCORES, detect_race_conditions=False)
    A_d = nc.declare_dram_parameter("A", [120, ROW], BF16, isOutput=False)
    v0f_d = nc.declare_dram_parameter("v0f", [120, 3], BF16, isOutput=False)
    v0b_d = nc.declare_dram_parameter("v0b", [120, 3], BF16, isOutput=False)
    B1_d = nc.declare_dram_parameter("B1", [120, 128], BF16, isOutput=False)
    outf_d = nc.declare_dram_parameter("voutf", [120, 3], BF16, isOutput=True)
    outb_d = nc.declare_dram_parameter("voutb", [120, 3], BF16, isOutput=True)

    add = mybir.AluOpType.add
    X = mybir.AxisListType.X
    import contextlib

    es = contextlib.ExitStack()
    with es:
        Araw = es.enter_context(nc.sbuf_tensor("Araw", [120, ROW], BF16))
        Ee = es.enter_context(nc.sbuf_tensor("Ee", [120, ROW], BF16))
        ea_f = es.enter_context(nc.sbuf_tensor("ea_f", [120, 60], BF16))
        ea_b = es.enter_context(nc.sbuf_tensor("ea_b", [120, 60], BF16))
        e2_f = es.enter_context(nc.sbuf_tensor("e2_f", [120, 60], BF16))
        e2_b = es.enter_context(nc.sbuf_tensor("e2_b", [120, 60], BF16))
        v_f = es.enter_context(nc.sbuf_tensor("v_f", [120, 3], BF16))
        v_b = es.enter_context(nc.sbuf_tensor("v_b", [120, 3], BF16))
        B1_sb = es.enter_context(nc.sbuf_tensor("B1_sb", [120, 128], BF16))
        ps_f = [
            es.enter_context(nc.psum_tensor(f"ps_f{i}", [128, 60], F32))
            for i in range(2)
        ]
        ps_b = [
            es.enter_context(nc.psum_tensor(f"ps_b{i}", [128, 60], F32))
            for i in range(2)
        ]

        s_cd = es.enter_context(nc.semaphore("s_cd"))
        s_dma = es.enter_context(nc.semaphore("s_dma"))
        s_exp = es.enter_context(nc.semaphore("s_exp"))
        s_ef = es.enter_context(nc.semaphore("s_ef"))
        s_eb = es.enter_context(nc.semaphore("s_eb"))
        s_mmf = es.enter_context(nc.semaphore("s_mmf"))
        s_mmb = es.enter_context(nc.semaphore("s_mmb"))
        s_vf = es.enter_context(nc.semaphore("s_vf"))
        s_vb = es.enter_context(nc.semaphore("s_vb"))
        block = es.enter_context(nc.Block())

        @block.sync
        def _(sync):
            sync.dma_start(B1_sb[:, :], B1_d[:, :]).then_inc(s_cd, 16)
            sync.dma_start(v_f[:, :], v0f_d[:, :]).then_inc(s_cd, 16)
            sync.dma_start(v_b[:, :], v0b_d[:, :]).then_inc(s_cd, 16)
            for c, n in enumerate(CHUNKS):
                lo, hi = CHUNK_START[c] * 240, (CHUNK_START[c] + n) * 240
                if c > 0:
                    # serialize: chunk completions must inc s_dma in order
                    sync.wait_ge(s_dma, 16 * c)
                sync.dma_start(Araw[:, lo:hi], A_d[:, lo:hi]).then_inc(s_dma, 16)
            sync.wait_ge(s_vf, NB)
            sync.wait_ge(s_vb, NB)
            sync.dma_start(outf_d[:, :], v_f[:, :]).then_inc(s_dma, 16)
            sync.dma_start(outb_d[:, :], v_b[:, :]).then_inc(s_dma, 16)

        @block.scalar
        def _(scalar):
            for c, n in enumerate(CHUNKS):
                lo, hi = CHUNK_START[c] * 240, (CHUNK_START[c] + n) * 240
                scalar.wait_ge(s_dma, 16 * (c + 1))
                scalar.activation(
                    Ee[:, lo:hi],
                    Araw[:, lo:hi],
                    mybir.ActivationFunctionType.Exp,
                ).then_inc(s_exp, 1)

        @block.tensor
        def _(tensor):
            tensor.wait_ge(s_cd, 48)
            for r in range(NB):
                # psum double-buffered: bank r%2 was last read by om(r-2),
                # which precedes em(r) in DVE program order -> no WAR wait.
                tensor.wait_ge(s_ef, r + 1)
                tensor.matmul(
                    ps_f[r % 2][:, :], B1_sb[:, :], ea_f[:, :], start=True, stop=True
                ).then_inc(s_mmf, 1)
                tensor.wait_ge(s_eb, r + 1)
                tensor.matmul(
                    ps_b[r % 2][:, :], B1_sb[:, :], ea_b[:, :], start=True, stop=True
                ).then_inc(s_mmb, 1)

        @block.vector
        def _(vector):
            vector.wait_ge(s_cd, 48)
            vbcf = bass.AP(v_f, 0, [[3, 120], [1, 3], [0, 20]])
            vbcb = bass.AP(v_b, 0, [[3, 120], [1, 3], [0, 20]])
            rof = bass.AP(v_f, 0, [[3, 120], [1, 3]])
            rob = bass.AP(v_b, 0, [[3, 120], [1, 3]])
            rif = bass.AP(e2_f, 0, [[60, 120], [20, 3], [1, 20]])
            rib = bass.AP(e2_b, 0, [[60, 120], [20, 3], [1, 20]])
            psf120 = [bass.AP(p, 0, [[60, 120], [1, 60]]) for p in ps_f]
            psb120 = [bass.AP(p, 0, [[60, 120], [1, 60]]) for p in ps_b]

            def ap(r, eo, s):
                return bass.AP(
                    Ee, r * 240 + (eo * 2 + s) * 60, [[ROW, 120], [20, 3], [1, 20]]
                )

            lp = nc.allow_low_precision(
                reason="bf16 state validated: loss rel err ~1e-4 vs 2e-2 gate"
            )
            with lp:
                for r in range(NB):
                    if r == 0 or R2C[r] != R2C[r - 1]:
                        vector.wait_ge(s_exp, R2C[r] + 1)
                    vector.tensor_mul(ea_f[:, :], ap(r, 0, 0), vbcf).then_inc(s_ef, 1)
                    if r > 0:
                        vector.wait_ge(s_mmb, r)
                        vector.tensor_mul(e2_b[:, :], ap(r - 1, 1, 1), psb120[(r - 1) % 2])
                        vector.tensor_reduce(rob, rib, axis=X, op=add).then_inc(s_vb, 1)
                    vector.tensor_mul(ea_b[:, :], ap(r, 0, 1), vbcb).then_inc(s_eb, 1)
                    vector.wait_ge(s_mmf, r + 1)
                    vector.tensor_mul(e2_f[:, :], ap(r, 1, 0), psf120[r % 2])
                    vector.tensor_reduce(rof, rif, axis=X, op=add).then_inc(s_vf, 1)
                # drain: last bwd odd step
                vector.wait_ge(s_mmb, NB)
                vector.tensor_mul(e2_b[:, :], ap(NB - 1, 1, 1), psb120[(NB - 1) % 2])
                vector.tensor_reduce(rob, rib, axis=X, op=add).then_inc(s_vb, 1)

    return nc


_NC_CACHE = None


def _get_nc():
    global _NC_CACHE
    if _NC_CACHE is None:
        _NC_CACHE = _build_bass()
    return _NC_CACHE


def kernel(trellis, gold, mask, corpus_mask):
    trellis = np.asarray(trellis, dtype=np.float32)
    gold = np.asarray(gold).astype(np.int64)
    mask = np.asarray(mask).astype(bool)

    # ---- host: gold energy (O(L*B) gather) + per-b unmasked counts ----
    flat = trellis.reshape(L, B, T * T)
    energy = np.take_along_axis(flat, gold, axis=2)[..., 0]
    gold_energy = float(np.sum(np.where(mask, energy.astype(np.float64), 0.0)))
    cnt = mask[1:].sum(axis=0).astype(np.float64)

    # ---- host: fold chem cost, mask->identity, -kappa into log trellis ----
    chem = np.zeros((T,), np.float32)
    chem[:4] = 1.0
    lnI = np.full((T, T), np.float32(NEG))
    np.fill_diagonal(lnI, 0.0)
    Tt = trellis + (chem - np.float32(KAPPA))[None, None, None, :]
    Tt = np.where(mask[:, :, None, None], Tt, lnI[None, None])

    B1 = np.zeros((120, 128), NPBF16)
    for k in range(120):
        B1[k, (k // T) * T : (k // T + 1) * T] = 1.0

    be_real = [512 - 2 * m for m in range(1, NB)]  # BE without the m=0 pad

    in_maps = []
    for cidx in range(NCORES):
        A6 = np.empty((120, NB, 2, 2, 3, 20), np.float32)
        for (g, t), b_loc in SLOTS.items():
            bg = cidx * BS + b_loc
            rows = slice(20 * g, 20 * g + 20)
            A6[rows, :, 0, 0, t, :] = Tt[FE, bg].transpose(1, 0, 2)
            A6[rows, 0, 0, 1, t, :] = lnI
            A6[rows, 1:, 0, 1, t, :] = Tt[be_real, bg].transpose(2, 0, 1)
            A6[rows, :, 1, 0, t, :] = Tt[FO, bg].transpose(2, 0, 1)
            A6[rows, :, 1, 1, t, :] = Tt[BO, bg].transpose(1, 0, 2)
        for t in range(3):
            for g in range(6):
                if (g, t) in SLOTS:
                    continue
                A6[20 * g : 20 * g + 20, :, :, :, t, :] = lnI[:, None, None, None, :]
        A = np.ascontiguousarray(A6.reshape(120, ROW)).astype(NPBF16)

        v0 = np.zeros((120, 2, 3), np.float32)
        for (g, t), b_loc in SLOTS.items():
            bg = cidx * BS + b_loc
            v0[20 * g : 20 * g + 20, 0, t] = np.exp(trellis[0, bg, START, :])
            v0[20 * g + END, 1, t] = 1.0
        in_maps.append(
            {
                "A": A,
                "v0f": v0[:, 0].astype(NPBF16),
                "v0b": v0[:, 1].astype(NPBF16),
                "B1": B1,
            }
        )

    nc = _get_nc()
    global LAST_RESULT
    kw = dict(trace=True, tmpdir=PROFILE_DIR) if PROFILE_DIR else {}
    LAST_RESULT = run_bass_kernel_spmd(nc, in_maps, list(range(NCORES)), **kw)
    res = LAST_RESULT.results

    partition = 0.0
    for cidx in range(NCORES):
        vf = np.asarray(res[cidx]["voutf"], dtype=np.float64)
        vb = np.asarray(res[cidx]["voutb"], dtype=np.float64)
        for (g, t), b_loc in SLOTS.items():
            bg = cidx * BS + b_loc
            rows = slice(20 * g, 20 * g + 20)
            Z = float((vf[rows, t] * vb[rows, t]).sum())
            partition += np.log(Z) + KAPPA * cnt[bg]
    return np.float32((partition - gold_energy) / B)


# revision 21
# speedup vs baseline: 5.3563x; 1.0587x over previous
"""CRF loss kernel for Trainium2 (8 NeuronCores, batch-sharded).

Forward algorithm in exp space, v2 "skewed two-chain pipeline":
  * meet-in-the-middle: fwd chain (steps 1..256) + bwd chain (511..257 + one
    identity pad). The two chains run as separate FD=60 instruction streams,
    phase-shifted by half a round so each chain's PE matmul (group-sum)
    latency is hidden behind the other chain's DVE ops.
  * even/odd layout alternation per chain: even step = DVE mul (state
    broadcast over 'a') + PE block-ones matmul (partition group-sum,
    replicated along free); odd step = DVE mul against PSUM + free-axis
    reduce back to partition layout.
  * bf16 operands everywhere (fp32 PSUM accumulate); B1 padded to 128
    weight columns (fast weight load).

DVE round r:  em_f(r) | om_b(r-1) red_b(r-1) | em_b(r) | om_f(r) red_f(r)
PE  round r:  MM_f(r) (after em_f) , MM_b(r) (after em_b)
Gold energy + final log/dot: O(L*B) host work on tiny tensors.
"""
import numpy as np
import ml_dtypes

import concourse.bass as bass
import concourse.mybir as mybir
from concourse.bass_utils import run_bass_kernel_spmd

T = 20
START, END = 17, 18
KAPPA = 3.7881286
L, B = 512, 128
NCORES = 8
BS = B // NCORES
NEG = -100.0
F32 = mybir.dt.float32
BF16 = mybir.dt.bfloat16
NPBF16 = ml_dtypes.bfloat16
NB = 128          # rounds; each consumes 4 trellis steps (2 fwd + 2 bwd)
ROW = NB * 240    # free elems per partition of A: 240 per round
# geometric chunk ramp (rounds per chunk): small first chunk so the first
# exp + first compute round start ASAP; later chunks amortize dispatch cost
CHUNKS = [2, 2, 4, 8, 16, 24, 24, 24, 24]
assert sum(CHUNKS) == NB
CHUNK_START = [sum(CHUNKS[:i]) for i in range(len(CHUNKS))]
# round r -> index of chunk containing it
R2C = []
for _ci, _n in enumerate(CHUNKS):
    R2C += [_ci] * _n
PROFILE_DIR = None
LAST_RESULT = None

# slot map: (g in 0..5, t in 0..2) -> local batch index; t=2 uses groups 0..3
SLOTS = {}
for _t in range(3):
    for _g in range(6):
        if _t == 2 and _g >= 4:
            continue
        SLOTS[(_g, _t)] = (6 * _t + _g) if _t < 2 else (12 + _g)

FE = [2 * m + 1 for m in range(NB)]               # fwd even steps
FO = [2 * m + 2 for m in range(NB)]               # fwd odd steps
BE = [None] + [512 - 2 * m for m in range(1, NB)]  # bwd even (None = pad I)
BO = [511 - 2 * m for m in range(NB)]             # bwd odd steps


def _build_bass():
    nc = bass.Bass("TRN2", num_devices=NCORES, detect_race_conditions=False)
    A_d = nc.declare_dram_parameter("A", [120, ROW], BF16, isOutput=False)
    v0f_d = nc.declare_dram_parameter("v0f", [120, 3], BF16, isOutput=False)
    v0b_d = nc.declare_dram_parameter("v0b", [120, 3], BF16, isOutput=False)
    B1_d = nc.declare_dram_parameter("B1", [128, 128], BF16, isOutput=False)
    outf_d = nc.declare_dram_parameter("voutf", [120, 3], BF16, isOutput=True)
    outb_d = nc.declare_dram_parameter("voutb", [120, 3], BF16, isOutput=True)

    add = mybir.AluOpType.add
    X = mybir.AxisListType.X
    import contextlib

    es = contextlib.ExitStack()
    with es:
        Araw = es.enter_context(nc.sbuf_tensor("Araw", [120, ROW], BF16))
        Ee = es.enter_context(nc.sbuf_tensor("Ee", [120, ROW], BF16))
        ea_f = es.enter_context(nc.sbuf_tensor("ea_f", [128, 60], BF16))
        ea_b = es.enter_context(nc.sbuf_tensor("ea_b", [128, 60], BF16))
        e2_f = es.enter_context(nc.sbuf_tensor("e2_f", [120, 60], BF16))
        e2_b = es.enter_context(nc.sbuf_tensor("e2_b", [120, 60], BF16))
        v_f = es.enter_context(nc.sbuf_tensor("v_f", [120, 3], BF16))
        v_b = es.enter_context(nc.sbuf_tensor("v_b", [120, 3], BF16))
        B1_sb = es.enter_context(nc.sbuf_tensor("B1_sb", [128, 128], BF16))
        ps_f = [
            es.enter_context(nc.psum_tensor(f"ps_f{i}", [128, 60], F32))
            for i in range(2)
        ]
        ps_b = [
            es.enter_context(nc.psum_tensor(f"ps_b{i}", [128, 60], F32))
            for i in range(2)
        ]

        s_cd = es.enter_context(nc.semaphore("s_cd"))
        s_dma = es.enter_context(nc.semaphore("s_dma"))
        s_exp = es.enter_context(nc.semaphore("s_exp"))
        s_ef = es.enter_context(nc.semaphore("s_ef"))
        s_eb = es.enter_context(nc.semaphore("s_eb"))
        s_mmf = es.enter_context(nc.semaphore("s_mmf"))
        s_mmb = es.enter_context(nc.semaphore("s_mmb"))
        s_vf = es.enter_context(nc.semaphore("s_vf"))
        s_vb = es.enter_context(nc.semaphore("s_vb"))
        block = es.enter_context(nc.Block())

        @block.sync
        def _(sync):
            sync.dma_start(B1_sb[:, :], B1_d[:, :]).then_inc(s_cd, 16)
            sync.dma_start(v_f[:, :], v0f_d[:, :]).then_inc(s_cd, 16)
            sync.dma_start(v_b[:, :], v0b_d[:, :]).then_inc(s_cd, 16)
            for c, n in enumerate(CHUNKS):
                lo, hi = CHUNK_START[c] * 240, (CHUNK_START[c] + n) * 240
                if c > 0:
                    # serialize: chunk completions must inc s_dma in order
                    sync.wait_ge(s_dma, 16 * c)
                sync.dma_start(Araw[:, lo:hi], A_d[:, lo:hi]).then_inc(s_dma, 16)
            sync.wait_ge(s_vf, NB)
            sync.wait_ge(s_vb, NB)
            sync.dma_start(outf_d[:, :], v_f[:, :]).then_inc(s_dma, 16)
            sync.dma_start(outb_d[:, :], v_b[:, :]).then_inc(s_dma, 16)

        @block.scalar
        def _(scalar):
            for c, n in enumerate(CHUNKS):
                lo, hi = CHUNK_START[c] * 240, (CHUNK_START[c] + n) * 240
                scalar.wait_ge(s_dma, 16 * (c + 1))
                scalar.activation(
                    Ee[:, lo:hi],
                    Araw[:, lo:hi],
                    mybir.ActivationFunctionType.Exp,
                ).then_inc(s_exp, 1)

        @block.tensor
        def _(tensor):
            tensor.wait_ge(s_cd, 48)
            eaf128 = bass.AP(ea_f, 0, [[60, 128], [1, 60]])
            eab128 = bass.AP(ea_b, 0, [[60, 128], [1, 60]])
            for r in range(NB):
                # psum double-buffered: bank r%2 was last read by om(r-2),
                # which precedes em(r) in DVE program order -> no WAR wait.
                tensor.wait_ge(s_ef, r + 1)
                tensor.matmul(
                    ps_f[r % 2][:, :], B1_sb[:, :], eaf128, start=True, stop=True
                ).then_inc(s_mmf, 1)
                tensor.wait_ge(s_eb, r + 1)
                tensor.matmul(
                    ps_b[r % 2][:, :], B1_sb[:, :], eab128, start=True, stop=True
                ).then_inc(s_mmb, 1)

        @block.vector
        def _(vector):
            vector.wait_ge(s_cd, 48)
            # zero ea once: the 8 contraction-pad rows must not be NaN
            # (0-weights, but NaN*0=NaN); rows 0-119 overwritten every round
            vector.memset(ea_f[:, :], 0.0)
            vector.memset(ea_b[:, :], 0.0)
            eaf120 = bass.AP(ea_f, 0, [[60, 120], [1, 60]])
            eab120 = bass.AP(ea_b, 0, [[60, 120], [1, 60]])
            vbcf = bass.AP(v_f, 0, [[3, 120], [1, 3], [0, 20]])
            vbcb = bass.AP(v_b, 0, [[3, 120], [1, 3], [0, 20]])
            rof = bass.AP(v_f, 0, [[3, 120], [1, 3]])
            rob = bass.AP(v_b, 0, [[3, 120], [1, 3]])
            rif = bass.AP(e2_f, 0, [[60, 120], [20, 3], [1, 20]])
            rib = bass.AP(e2_b, 0, [[60, 120], [20, 3], [1, 20]])
            psf120 = [bass.AP(p, 0, [[60, 120], [1, 60]]) for p in ps_f]
            psb120 = [bass.AP(p, 0, [[60, 120], [1, 60]]) for p in ps_b]

            def ap(r, eo, s):
                return bass.AP(
                    Ee, r * 240 + (eo * 2 + s) * 60, [[ROW, 120], [20, 3], [1, 20]]
                )

            lp = nc.allow_low_precision(
                reason="bf16 state validated: loss rel err ~1e-4 vs 2e-2 gate"
            )
            with lp:
                for r in range(NB):
                    if r == 0 or R2C[r] != R2C[r - 1]:
                        vector.wait_ge(s_exp, R2C[r] + 1)
                    vector.tensor_mul(eaf120, ap(r, 0, 0), vbcf).then_inc(s_ef, 1)
                    if r > 0:
                        vector.wait_ge(s_mmb, r)
                        vector.tensor_mul(e2_b[:, :], ap(r - 1, 1, 1), psb120[(r - 1) % 2])
                        vector.tensor_reduce(rob, rib, axis=X, op=add).then_inc(s_vb, 1)
                    vector.tensor_mul(eab120, ap(r, 0, 1), vbcb).then_inc(s_eb, 1)
                    vector.wait_ge(s_mmf, r + 1)
                    vector.tensor_mul(e2_f[:, :], ap(r, 1, 0), psf120[r % 2])
                    vector.tensor_reduce(rof, rif, axis=X, op=add).then_inc(s_vf, 1)
                # drain: last bwd odd step
                vector.wait_ge(s_mmb, NB)
                vector.tensor_mul(e2_b[:, :], ap(NB - 1, 1, 1), psb120[(NB - 1) % 2])
                vector.tensor_reduce(rob, rib, axis=X, op=add).then_inc(s_vb, 1)

    return nc


_NC_CACHE = None


def _get_nc():
    global _NC_CACHE
    if _NC_CACHE is None:
        _NC_CACHE = _build_bass()
    return _NC_CACHE


def kernel(trellis, gold, mask, corpus_mask):
    trellis = np.asarray(trellis, dtype=np.float32)
    gold = np.asarray(gold).astype(np.int64)
    mask = np.asarray(mask).astype(bool)

    # ---- host: gold energy (O(L*B) gather) + per-b unmasked counts ----
    flat = trellis.reshape(L, B, T * T)
    energy = np.take_along_axis(flat, gold, axis=2)[..., 0]
    gold_energy = float(np.sum(np.where(mask, energy.astype(np.float64), 0.0)))
    cnt = mask[1:].sum(axis=0).astype(np.float64)

    # ---- host: fold chem cost, mask->identity, -kappa into log trellis ----
    chem = np.zeros((T,), np.float32)
    chem[:4] = 1.0
    lnI = np.full((T, T), np.float32(NEG))
    np.fill_diagonal(lnI, 0.0)
    Tt = trellis + (chem - np.float32(KAPPA))[None, None, None, :]
    Tt = np.where(mask[:, :, None, None], Tt, lnI[None, None])

    B1 = np.zeros((128, 128), NPBF16)
    for k in range(120):
        B1[k, (k // T) * T : (k // T + 1) * T] = 1.0

    be_real = [512 - 2 * m for m in range(1, NB)]  # BE without the m=0 pad

    in_maps = []
    for cidx in range(NCORES):
        A6 = np.empty((120, NB, 2, 2, 3, 20), np.float32)
        for (g, t), b_loc in SLOTS.items():
            bg = cidx * BS + b_loc
            rows = slice(20 * g, 20 * g + 20)
            A6[rows, :, 0, 0, t, :] = Tt[FE, bg].transpose(1, 0, 2)
            A6[rows, 0, 0, 1, t, :] = lnI
            A6[rows, 1:, 0, 1, t, :] = Tt[be_real, bg].transpose(2, 0, 1)
            A6[rows, :, 1, 0, t, :] = Tt[FO, bg].transpose(2, 0, 1)
            A6[rows, :, 1, 1, t, :] = Tt[BO, bg].transpose(1, 0, 2)
        for t in range(3):
            for g in range(6):
                if (g, t) in SLOTS:
                    continue
                A6[20 * g : 20 * g + 20, :, :, :, t, :] = lnI[:, None, None, None, :]
        A = np.ascontiguousarray(A6.reshape(120, ROW)).astype(NPBF16)

        v0 = np.zeros((120, 2, 3), np.float32)
        for (g, t), b_loc in SLOTS.items():
            bg = cidx * BS + b_loc
            v0[20 * g : 20 * g + 20, 0, t] = np.exp(trellis[0, bg, START, :])
            v0[20 * g + END, 1, t] = 1.0
        in_maps.append(
            {
                "A": A,
                "v0f": v0[:, 0].astype(NPBF16),
                "v0b": v0[:, 1].astype(NPBF16),
                "B1": B1,
            }
        )

    nc = _get_nc()
    global LAST_RESULT
    kw = dict(trace=True, tmpdir=PROFILE_DIR) if PROFILE_DIR else {}
    LAST_RESULT = run_bass_kernel_spmd(nc, in_maps, list(range(NCORES)), **kw)
    res = LAST_RESULT.results

    partition = 0.0
    for cidx in range(NCORES):
        vf = np.asarray(res[cidx]["voutf"], dtype=np.float64)
        vb = np.asarray(res[cidx]["voutb"], dtype=np.float64)
        for (g, t), b_loc in SLOTS.items():
            bg = cidx * BS + b_loc
            rows = slice(20 * g, 20 * g + 20)
            Z = float((vf[rows, t] * vb[rows, t]).sum())
            partition += np.log(Z) + KAPPA * cnt[bg]
    return np.float32((partition - gold_energy) / B)
